# revision 7
# baseline (speedup 1.0000x reference)
"""Multi-head attention (b=4, n=2048, dim=1024, heads=16) on 8 TRN2 cores.

Sharding: tensor-parallel over heads (2 heads per core) + row-parallel output
projection; host sums the 8 partial outputs and adds the bias.

Per-core math (heads h0=2c, h1=2c+1):
  qkv^T = w_in_c^T @ x^T          (f32r matmuls, w stationary)
  S^T   = k_h^T.T @ q_h^T         (bf16, K=64, per-head partition halves)
  E^T   = exp(S^T / 8)            (ACT, no max subtraction: scores ~ N(0,1))
  [o^T; denom] = [v_h | 1].T @ E^T  (bf16, M=65 fuses softmax denominator)
  o_norm^T = o^T * (1/denom)      (DVE + DMA partition-broadcast)
  partial^T = w_out_c.T @ o_norm^T  (bf16, w stationary) -> DRAM
"""

import os
import sys
import types

import numpy as np

# NTFF-profile hook shim: container's antenv lacks axon_hooks; harmless if
# tracing is never requested.
if "antenv.axon_hooks" not in sys.modules:
    try:
        from trn_agent_boot.trn_boot import _ntff_profile_via_ctypes

        _m = types.ModuleType("antenv.axon_hooks")
        _h = _ntff_profile_via_ctypes("/opt/axon/libaxon_pjrt.so")
        _m.get_axon_ntff_profile_hook = lambda: _h
        _m.set_axon_ntff_profile_hook = lambda hook: None
        sys.modules["antenv.axon_hooks"] = _m
    except Exception:
        pass

import ml_dtypes

import concourse.bacc as bacc
import concourse.bass as bass
import concourse.mybir as mybir
import concourse.tile as tile
from concourse.bass_utils import run_bass_kernel_spmd
from concourse.masks import make_identity

F32 = mybir.dt.float32
F32R = mybir.dt.float32r
BF16 = mybir.dt.bfloat16

B, N, DIM, HEADS = 4, 2048, 1024, 16
HD = DIM // HEADS          # 64
NCORES = 8
HPC = HEADS // NCORES      # 2 heads per core
NT = B * N                 # 8192 tokens
MQKV = 3 * HPC * HD        # 384 qkv output dims per core
SCALE = HD ** -0.5         # 0.125

KT_TILES = DIM // 128      # 8 k-tiles in the projection contraction
NB = NT // 512             # 16 n-blocks in phase 1
JT = N // 128              # 16 j-tiles per batch
IH = N // 1024             # 2 i-halves per batch


def _build_nc():
    nc = bacc.Bacc("TRN2", target_bir_lowering=False, debug=False)

    xT = nc.dram_tensor("xT", [DIM, NT], F32R, kind="ExternalInput")
    w_in_c = nc.dram_tensor("w_in_c", [DIM, MQKV], F32R, kind="ExternalInput")
    w_out_c = nc.dram_tensor("w_out_c", [128, DIM], BF16, kind="ExternalInput")
    po = nc.dram_tensor("po", [DIM, NT], F32, kind="ExternalOutput")

    with tile.TileContext(nc) as tc:
        with (
            tc.tile_pool(name="big", bufs=1) as big,
            tc.tile_pool(name="strm", bufs=2) as strm,
            tc.tile_pool(name="et", bufs=4) as etp,
            tc.tile_pool(name="ps", bufs=3, space="PSUM") as ps,
        ):
            # ---- persistent SBUF ----
            QT = big.tile([128, NT], BF16)    # [q_h0(0:64); q_h1(64:128)]^T
            KT = big.tile([128, NT], BF16)
            Vt = big.tile([128, B * JT, 130], BF16)  # [v_h0|1|v_h1|1] per j-tile
            o_sb = big.tile([128, NT], BF16)  # normalized o^T, both heads
            w_in_sb = big.tile([128, KT_TILES, MQKV], F32R)
            w_out_sb = big.tile([128, DIM], BF16)

            nc.sync.dma_start(
                out=w_in_sb,
                in_=w_in_c.rearrange("(kt p) m -> p kt m", p=128),
            )
            nc.sync.dma_start(out=w_out_sb, in_=w_out_c[:, :])
            ident = big.tile([128, 128], BF16)
            make_identity(nc, ident)
            ones_f = big.tile([128, 64], F32)
            nc.vector.memset(ones_f, 1.0)
            ones_t = big.tile([128, 64], F32R)
            nc.vector.tensor_copy(ones_t, ones_f)
            nc.vector.memset(Vt[:, :, 64], 1.0)
            nc.vector.memset(Vt[:, :, 129], 1.0)

            xT_r = xT.rearrange("(kt p) n -> p kt n", p=128)

            # ================= Phase 1: QKV projection =================
            for nb in range(NB):
                ncol = slice(nb * 512, (nb + 1) * 512)
                xin = strm.tile([128, KT_TILES, 512], F32R, tag="xin")
                for k in range(KT_TILES):
                    nc.sync.dma_start(out=xin[:, k, :], in_=xT_r[:, k, ncol])
                for m in range(3):
                    pj = ps.tile([128, 512], F32, tag="ps_s", name=f"pj{nb}_{m}")
                    for k in range(KT_TILES):
                        nc.tensor.matmul(
                            pj,
                            w_in_sb[:, k, m * 128:(m + 1) * 128],
                            xin[:, k, :],
                            start=(k == 0),
                            stop=(k == KT_TILES - 1),
                        )
                    if m == 0:
                        nc.vector.tensor_copy(QT[:, ncol], pj)
                    elif m == 1:
                        nc.vector.tensor_copy(KT[:, ncol], pj)
                    else:
                        vstage = strm.tile([128, 512], BF16, tag="vstage")
                        nc.vector.tensor_copy(vstage, pj)
                        for c in range(4):
                            g = nb * 4 + c
                            tp = ps.tile(
                                [128, 128], BF16, tag="ps_s", name=f"tp{g}"
                            )
                            nc.tensor.transpose(
                                tp, vstage[:, c * 128:(c + 1) * 128], ident
                            )
                            nc.vector.tensor_copy(Vt[:, g, 0:64], tp[:, 0:64])
                            nc.vector.tensor_copy(Vt[:, g, 65:129], tp[:, 64:128])

            # ================= Phase 2: attention =================
            for b in range(B):
                for h in range(HPC):
                    hp = slice(h * 64, (h + 1) * 64)   # partition rows of head
                    vcol = slice(h * 65, h * 65 + 65)  # Vt cols [v_h | 1]
                    for ihalf in range(IH):
                        icol = slice(
                            b * N + ihalf * 1024, b * N + ihalf * 1024 + 1024
                        )
                        po_t = ps.tile(
                            [65, 1024], F32, tag="ps_o", bufs=1,
                            name=f"po{b}_{h}_{ihalf}",
                        )
                        for jt in range(JT):
                            jcol = slice(b * N + jt * 128, b * N + jt * 128 + 128)
                            st = ps.tile(
                                [128, 1024], F32, tag="ps_s",
                                name=f"st{b}_{h}_{ihalf}_{jt}",
                            )
                            for q2 in range(2):
                                i0 = b * N + ihalf * 1024 + q2 * 512
                                nc.tensor.matmul(
                                    st[:, q2 * 512:(q2 + 1) * 512],
                                    KT[hp, jcol],
                                    QT[hp, i0:i0 + 512],
                                    start=True, stop=True,
                                )
                            et = etp.tile([128, 1024], BF16, tag="et", name="et")
                            nc.scalar.activation(
                                et, st, mybir.ActivationFunctionType.Exp,
                                scale=SCALE,
                            )
                            for q2 in range(2):
                                nc.tensor.matmul(
                                    po_t[:, q2 * 512:(q2 + 1) * 512],
                                    Vt[:, b * JT + jt, vcol],
                                    et[:, q2 * 512:(q2 + 1) * 512],
                                    start=(jt == 0), stop=(jt == JT - 1),
                                )
                        # normalize: rows 0:64 are o^T, row 64 is the denom
                        ost = strm.tile([65, 1024], F32R, tag="ost")
                        nc.vector.tensor_copy(ost, po_t)
                        with nc.allow_low_precision(
                            reason="f32r recip feeds f32r broadcast matmul"
                        ):
                            nc.vector.reciprocal(ost[64:65, :], ost[64:65, :])
                        # broadcast recip row to 64 partitions via K=1 matmul
                        bps = ps.tile([64, 1024], F32, tag="ps_s", name="bps")
                        for half in range(2):
                            hs = slice(half * 512, (half + 1) * 512)
                            nc.tensor.matmul(
                                bps[:, hs], ones_t[64:65, :], ost[64:65, hs],
                                start=True, stop=True,
                            )
                        if h == 0:
                            nc.vector.tensor_mul(
                                o_sb[0:64, icol], ost[0:64, :], bps
                            )
                        else:
                            h1s = strm.tile([64, 1024], BF16, tag="h1s")
                            nc.vector.tensor_mul(h1s, ost[0:64, :], bps)
                            nc.sync.dma_start(out=o_sb[64:128, icol], in_=h1s)

            # ================= Phase 3: output projection =================
            for mt in range(DIM // 128):
                for nb2 in range(NT // 1024):
                    pp = ps.tile(
                        [128, 1024], F32, tag="ps_s", name=f"pp{mt}_{nb2}"
                    )
                    for q2 in range(2):
                        nc.tensor.matmul(
                            pp[:, q2 * 512:(q2 + 1) * 512],
                            w_out_sb[:, mt * 128:(mt + 1) * 128],
                            o_sb[
                                :,
                                nb2 * 1024 + q2 * 512:nb2 * 1024 + (q2 + 1) * 512,
                            ],
                            start=True, stop=True,
                        )
                    pout = strm.tile([128, 1024], F32, tag="pout")
                    nc.vector.tensor_copy(pout, pp)
                    nc.sync.dma_start(
                        out=po[
                            mt * 128:(mt + 1) * 128,
                            nb2 * 1024:(nb2 + 1) * 1024,
                        ],
                        in_=pout,
                    )

    nc.finalize()
    return nc


_CACHED = {}


def kernel(x, w_in, w_out, b_out, _trace=False):
    if "nc" not in _CACHED:
        _CACHED["nc"] = _build_nc()
    nc = _CACHED["nc"]

    x2 = np.ascontiguousarray(
        x.reshape(NT, DIM).T.astype(np.float32)
    )  # [DIM, NT]
    in_maps = []
    for c in range(NCORES):
        h0, h1 = HPC * c, HPC * c + 1
        cols = []
        for part in range(3):  # q, k, v
            base = part * DIM
            cols.extend(range(base + h0 * HD, base + h0 * HD + HD))
            cols.extend(range(base + h1 * HD, base + h1 * HD + HD))
        w_in_c = np.ascontiguousarray(w_in[:, cols].astype(np.float32))
        w_out_c = np.ascontiguousarray(
            w_out[128 * c:128 * (c + 1), :].astype(np.float32)
        )
        in_maps.append(
            {
                "xT": x2,
                "w_in_c": w_in_c,
                "w_out_c": w_out_c.astype(ml_dtypes.bfloat16),
            }
        )

    res = run_bass_kernel_spmd(
        nc, in_maps, core_ids=list(range(NCORES)), trace=_trace
    )
    acc = res.results[0]["po"].astype(np.float64)
    for c in range(1, NCORES):
        acc = acc + res.results[c]["po"].astype(np.float64)
    out = acc.T + b_out.astype(np.float64)
    if _trace:
        kernel.last_result = res
    return np.ascontiguousarray(out.reshape(B, N, DIM).astype(np.float32))


# revision 10
# speedup vs baseline: 1.5255x; 1.5255x over previous
"""Multi-head attention (b=4, n=2048, dim=1024, heads=16) on 8 TRN2 cores.

Sharding: tensor-parallel over heads (2 heads per core) + row-parallel output
projection; host sums the 8 partial outputs and adds the bias.

Per-core math (heads h0=2c, h1=2c+1):
  qkv^T = w_in_c^T @ x^T          (f32r matmuls, w stationary)
  S^T   = k_h^T.T @ q_h^T         (bf16, K=64, per-head partition halves)
  E^T   = exp(S^T / 8)            (ACT, no max subtraction: scores ~ N(0,1))
  [o^T; denom] = [v_h | 1].T @ E^T  (bf16, M=65 fuses softmax denominator)
  o_norm^T = o^T * (1/denom)      (DVE + DMA partition-broadcast)
  partial^T = w_out_c.T @ o_norm^T  (bf16, w stationary) -> DRAM
"""

import os
import sys
import types

import numpy as np

# NTFF-profile hook shim: container's antenv lacks axon_hooks; harmless if
# tracing is never requested.
if "antenv.axon_hooks" not in sys.modules:
    try:
        from trn_agent_boot.trn_boot import _ntff_profile_via_ctypes

        _m = types.ModuleType("antenv.axon_hooks")
        _h = _ntff_profile_via_ctypes("/opt/axon/libaxon_pjrt.so")
        _m.get_axon_ntff_profile_hook = lambda: _h
        _m.set_axon_ntff_profile_hook = lambda hook: None
        sys.modules["antenv.axon_hooks"] = _m
    except Exception:
        pass

import ml_dtypes

import concourse.bacc as bacc
import concourse.bass as bass
import concourse.mybir as mybir
import concourse.tile as tile
from concourse.bass_utils import run_bass_kernel_spmd
from concourse.masks import make_identity

F32 = mybir.dt.float32
F32R = mybir.dt.float32r
BF16 = mybir.dt.bfloat16

B, N, DIM, HEADS = 4, 2048, 1024, 16
HD = DIM // HEADS          # 64
NCORES = 8
HPC = HEADS // NCORES      # 2 heads per core
NT = B * N                 # 8192 tokens
MQKV = 3 * HPC * HD        # 384 qkv output dims per core
SCALE = HD ** -0.5         # 0.125

KT_TILES = DIM // 128      # 8 k-tiles in the projection contraction
NB = NT // 512             # 16 n-blocks in phase 1
JT = N // 128              # 16 j-tiles per batch
IH = N // 1024             # 2 i-halves per batch


def _build_nc():
    nc = bacc.Bacc("TRN2", target_bir_lowering=False, debug=False)

    xT = nc.dram_tensor("xT", [DIM, NT], BF16, kind="ExternalInput")
    w_in_c = nc.dram_tensor("w_in_c", [DIM, MQKV], BF16, kind="ExternalInput")
    w_out_c = nc.dram_tensor("w_out_c", [128, DIM], BF16, kind="ExternalInput")
    po = nc.dram_tensor("po", [DIM, NT], F32, kind="ExternalOutput")
    dn_dram = nc.dram_tensor("dn_dram", [16, 1024], F32)
    rc_dram = nc.dram_tensor("rc_dram", [16, 1024], F32)

    with tile.TileContext(nc) as tc:
        with (
            tc.tile_pool(name="big", bufs=1) as big,
            tc.tile_pool(name="strm", bufs=2) as strm,
            tc.tile_pool(name="et", bufs=4) as etp,
            tc.tile_pool(name="ps", bufs=3, space="PSUM") as ps,
        ):
            # ---- persistent SBUF ----
            QT = big.tile([128, NT], BF16)    # [q_h0(0:64); q_h1(64:128)]^T
            KT = big.tile([128, NT], BF16)
            Vt = big.tile([128, B * JT, 130], BF16)  # [v_h0|1|v_h1|1] per j-tile
            o_sb = big.tile([128, NT], BF16)  # o^T both heads (normed in place)
            w_in_sb = big.tile([128, KT_TILES, MQKV], BF16)
            w_out_sb = big.tile([128, DIM], BF16)

            nc.sync.dma_start(
                out=w_in_sb,
                in_=w_in_c.rearrange("(kt p) m -> p kt m", p=128),
            )
            nc.sync.dma_start(out=w_out_sb, in_=w_out_c[:, :])
            ident = big.tile([128, 128], BF16)
            make_identity(nc, ident)
            nc.vector.memset(Vt[:, :, 64], 1.0)
            nc.vector.memset(Vt[:, :, 129], 1.0)

            xT_r = xT.rearrange("(kt p) n -> p kt n", p=128)

            # ================= Phase 1: QKV projection =================
            NB1 = NT // 1024
            for nb in range(NB1):
                ncol = slice(nb * 1024, (nb + 1) * 1024)
                xin = strm.tile([128, KT_TILES, 2, 512], BF16, tag="xin")
                for k in range(KT_TILES):
                    nc.sync.dma_start(
                        out=xin[:, k, :, :],
                        in_=xT_r[:, k, ncol].rearrange(
                            "p (a b) -> p a b", b=512
                        ),
                    )
                for m in range(3):
                    pj = ps.tile(
                        [128, 2, 512], F32, tag="ps_s", name=f"pj{nb}_{m}"
                    )
                    for k in range(KT_TILES):
                        for a in range(2):
                            nc.tensor.matmul(
                                pj[:, a, :],
                                w_in_sb[:, k, m * 128:(m + 1) * 128],
                                xin[:, k, a, :],
                                start=(k == 0),
                                stop=(k == KT_TILES - 1),
                            )
                    pjf = pj.rearrange("p a b -> p (a b)")
                    if m == 0:
                        nc.vector.tensor_copy(QT[:, ncol], pjf)
                    elif m == 1:
                        nc.vector.tensor_copy(KT[:, ncol], pjf)
                    else:
                        vstage = strm.tile([128, 1024], BF16, tag="vstage")
                        nc.vector.tensor_copy(vstage, pjf)
                        for c in range(8):
                            g = nb * 8 + c
                            tp = ps.tile(
                                [128, 128], BF16, tag="ps_s", name=f"tp{g}"
                            )
                            nc.tensor.transpose(
                                tp, vstage[:, c * 128:(c + 1) * 128], ident
                            )
                            nc.vector.tensor_copy(Vt[:, g, 0:64], tp[:, 0:64])
                            nc.vector.tensor_copy(
                                Vt[:, g, 65:129], tp[:, 64:128]
                            )

            # ================= Phase 2: attention =================
            for b in range(B):
                for h in range(HPC):
                    hp = slice(h * 64, (h + 1) * 64)   # partition rows of head
                    vcol = slice(h * 65, h * 65 + 65)  # Vt cols [v_h | 1]
                    for ihalf in range(IH):
                        seg = b * 4 + h * 2 + ihalf
                        i0 = b * N + ihalf * 1024
                        icol = slice(i0, i0 + 1024)
                        po_t = ps.tile(
                            [65, 2, 512], F32, tag="ps_o", bufs=1,
                            name=f"po{seg}",
                        )
                        for jt in range(JT):
                            jcol = slice(b * N + jt * 128, b * N + jt * 128 + 128)
                            st = ps.tile(
                                [128, 2, 512], F32, tag="ps_s", name=f"st{seg}_{jt}"
                            )
                            for a in range(2):
                                nc.tensor.matmul(
                                    st[:, a, :], KT[hp, jcol],
                                    QT[hp, i0 + a * 512:i0 + (a + 1) * 512],
                                    start=True, stop=True,
                                )
                            et = etp.tile([128, 1024], BF16, tag="et", name="et")
                            nc.scalar.activation(
                                et, st.rearrange("p a b -> p (a b)"),
                                mybir.ActivationFunctionType.Exp,
                                scale=SCALE,
                            )
                            for a in range(2):
                                nc.tensor.matmul(
                                    po_t[:, a, :], Vt[:, b * JT + jt, vcol],
                                    et[:, a * 512:(a + 1) * 512],
                                    start=(jt == 0), stop=(jt == JT - 1),
                                )
                        # drain psum: o rows -> o_sb (unnormalized), denom -> DRAM
                        po_f = po_t.rearrange("p a b -> p (a b)")
                        if h == 0:
                            nc.vector.tensor_copy(o_sb[0:64, icol], po_f[0:64, :])
                        else:
                            h1s = strm.tile([64, 1024], BF16, tag="h1s")
                            nc.vector.tensor_copy(h1s, po_f[0:64, :])
                            nc.sync.dma_start(out=o_sb[64:128, icol], in_=h1s)
                        dnst = strm.tile([1, 1024], F32, tag="dnst")
                        nc.vector.tensor_copy(dnst, po_f[64:65, :])
                        nc.sync.dma_start(out=dn_dram[seg:seg + 1, :], in_=dnst[0:1, :])

                # -------- normalize batch b (overlaps next batch) --------
                dns = strm.tile([4, 1024], F32, tag="dns")
                nc.sync.dma_start(out=dns, in_=dn_dram[b * 4:(b + 1) * 4, :])
                with nc.allow_low_precision(reason="denfor broadcast"):
                    nc.vector.reciprocal(dns, dns)
                nc.sync.dma_start(out=rc_dram[b * 4:(b + 1) * 4, :], in_=dns)
                for h in range(HPC):
                    for ihalf in range(IH):
                        seg = b * 4 + h * 2 + ihalf
                        i0 = b * N + ihalf * 1024
                        icol = slice(i0, i0 + 1024)
                        rows = slice(h * 64, (h + 1) * 64)
                        bcast = strm.tile([128, 1024], F32, tag="bcast")
                        src = rc_dram[seg:seg + 1, :]
                        rbc = bass.AP(
                            tensor=src.tensor,
                            offset=src.offset,
                            ap=[[0, 64]] + list(src.ap)[1:],
                        )
                        nc.sync.dma_start(out=bcast[rows, :], in_=rbc)
                        nc.vector.tensor_mul(
                            o_sb[rows, icol], o_sb[rows, icol], bcast[rows, :]
                        )

            # ================= Phase 3: output projection =================
            for mt in range(DIM // 128):
                for nb2 in range(NT // 1024):
                    pp = ps.tile(
                        [128, 2, 512], F32, tag="ps_s", name=f"pp{mt}_{nb2}"
                    )
                    for a in range(2):
                        nc.tensor.matmul(
                            pp[:, a, :],
                            w_out_sb[:, mt * 128:(mt + 1) * 128],
                            o_sb[
                                :,
                                nb2 * 1024 + a * 512:nb2 * 1024 + (a + 1) * 512,
                            ],
                            start=True, stop=True,
                        )
                    pout = strm.tile([128, 1024], F32, tag="pout")
                    nc.vector.tensor_copy(pout, pp.rearrange("p a b -> p (a b)"))
                    nc.sync.dma_start(
                        out=po[
                            mt * 128:(mt + 1) * 128,
                            nb2 * 1024:(nb2 + 1) * 1024,
                        ],
                        in_=pout,
                    )

    nc.finalize()
    return nc


_CACHED = {}


def kernel(x, w_in, w_out, b_out, _trace=False):
    if "nc" not in _CACHED:
        _CACHED["nc"] = _build_nc()
    nc = _CACHED["nc"]

    x2 = np.ascontiguousarray(
        x.reshape(NT, DIM).T.astype(np.float32)
    )  # [DIM, NT]
    in_maps = []
    for c in range(NCORES):
        h0, h1 = HPC * c, HPC * c + 1
        cols = []
        for part in range(3):  # q, k, v
            base = part * DIM
            cols.extend(range(base + h0 * HD, base + h0 * HD + HD))
            cols.extend(range(base + h1 * HD, base + h1 * HD + HD))
        w_in_c = np.ascontiguousarray(w_in[:, cols].astype(np.float32))
        w_out_c = np.ascontiguousarray(
            w_out[128 * c:128 * (c + 1), :].astype(np.float32)
        )
        in_maps.append(
            {
                "xT": x2.astype(ml_dtypes.bfloat16),
                "w_in_c": w_in_c.astype(ml_dtypes.bfloat16),
                "w_out_c": w_out_c.astype(ml_dtypes.bfloat16),
            }
        )

    res = run_bass_kernel_spmd(
        nc, in_maps, core_ids=list(range(NCORES)), trace=_trace
    )
    acc = res.results[0]["po"].astype(np.float64)
    for c in range(1, NCORES):
        acc = acc + res.results[c]["po"].astype(np.float64)
    out = acc.T + b_out.astype(np.float64)
    if _trace:
        kernel.last_result = res
    return np.ascontiguousarray(out.reshape(B, N, DIM).astype(np.float32))


# revision 11
# speedup vs baseline: 1.7258x; 1.1313x over previous
"""Multi-head attention (b=4, n=2048, dim=1024, heads=16) on 8 TRN2 cores.

Sharding: tensor-parallel over heads (2 heads per core) + row-parallel output
projection; host sums the 8 partial outputs and adds the bias.

Per-core math (heads h0=2c, h1=2c+1):
  qkv^T = w_in_c^T @ x^T          (f32r matmuls, w stationary)
  S^T   = k_h^T.T @ q_h^T         (bf16, K=64, per-head partition halves)
  E^T   = exp(S^T / 8)            (ACT, no max subtraction: scores ~ N(0,1))
  [o^T; denom] = [v_h | 1].T @ E^T  (bf16, M=65 fuses softmax denominator)
  o_norm^T = o^T * (1/denom)      (DVE + DMA partition-broadcast)
  partial^T = w_out_c.T @ o_norm^T  (bf16, w stationary) -> DRAM
"""

import os
import sys
import types

import numpy as np

# NTFF-profile hook shim: container's antenv lacks axon_hooks; harmless if
# tracing is never requested.
if "antenv.axon_hooks" not in sys.modules:
    try:
        from trn_agent_boot.trn_boot import _ntff_profile_via_ctypes

        _m = types.ModuleType("antenv.axon_hooks")
        _h = _ntff_profile_via_ctypes("/opt/axon/libaxon_pjrt.so")
        _m.get_axon_ntff_profile_hook = lambda: _h
        _m.set_axon_ntff_profile_hook = lambda hook: None
        sys.modules["antenv.axon_hooks"] = _m
    except Exception:
        pass

import ml_dtypes

import concourse.bacc as bacc
import concourse.bass as bass
import concourse.mybir as mybir
import concourse.tile as tile
from concourse.bass_utils import run_bass_kernel_spmd
from concourse.masks import make_identity

F32 = mybir.dt.float32
F32R = mybir.dt.float32r
BF16 = mybir.dt.bfloat16

B, N, DIM, HEADS = 4, 2048, 1024, 16
HD = DIM // HEADS          # 64
NCORES = 8
HPC = HEADS // NCORES      # 2 heads per core
NT = B * N                 # 8192 tokens
MQKV = 3 * HPC * HD        # 384 qkv output dims per core
SCALE = HD ** -0.5         # 0.125

KT_TILES = DIM // 128      # 8 k-tiles in the projection contraction
NB = NT // 512             # 16 n-blocks in phase 1
JT = N // 128              # 16 j-tiles per batch
IH = N // 1024             # 2 i-halves per batch


def _build_nc():
    nc = bacc.Bacc("TRN2", target_bir_lowering=False, debug=False)

    xT = nc.dram_tensor("xT", [DIM, NT], BF16, kind="ExternalInput")
    w_in_c = nc.dram_tensor("w_in_c", [DIM, MQKV], BF16, kind="ExternalInput")
    w_out_c = nc.dram_tensor("w_out_c", [128, DIM], BF16, kind="ExternalInput")
    po = nc.dram_tensor("po", [DIM, NT], F32, kind="ExternalOutput")
    dn_dram = nc.dram_tensor("dn_dram", [16, 1024], F32)
    rc_dram = nc.dram_tensor("rc_dram", [16, 1024], F32)

    with tile.TileContext(nc) as tc:
        with (
            tc.tile_pool(name="big", bufs=1) as big,
            tc.tile_pool(name="strm", bufs=2) as strm,
            tc.tile_pool(name="et", bufs=4) as etp,
            tc.tile_pool(name="ps", bufs=2, space="PSUM") as ps,
        ):
            # ---- persistent SBUF ----
            QT = big.tile([128, NT], BF16)    # [q_h0(0:64); q_h1(64:128)]^T
            KT = big.tile([128, NT], BF16)
            Vt = big.tile([128, B * JT, 130], BF16)  # [v_h0|1|v_h1|1] per j-tile
            o_sb = big.tile([128, NT], BF16)  # o^T both heads (normed in place)
            w_in_sb = big.tile([128, KT_TILES, MQKV], BF16)
            w_out_sb = big.tile([128, DIM], BF16)

            nc.sync.dma_start(
                out=w_in_sb,
                in_=w_in_c.rearrange("(kt p) m -> p kt m", p=128),
            )
            nc.sync.dma_start(out=w_out_sb, in_=w_out_c[:, :])
            ident = big.tile([128, 128], BF16)
            make_identity(nc, ident)
            nc.vector.memset(Vt[:, :, 64], 1.0)
            nc.vector.memset(Vt[:, :, 129], 1.0)

            xT_r = xT.rearrange("(kt p) n -> p kt n", p=128)

            # ================= Phase 1: QKV projection =================
            NB1 = NT // 1024
            for nb in range(NB1):
                ncol = slice(nb * 1024, (nb + 1) * 1024)
                xin = strm.tile([128, KT_TILES, 2, 512], BF16, tag="xin")
                for k in range(KT_TILES):
                    nc.sync.dma_start(
                        out=xin[:, k, :, :],
                        in_=xT_r[:, k, ncol].rearrange(
                            "p (a b) -> p a b", b=512
                        ),
                    )
                for m in range(3):
                    pj = ps.tile(
                        [128, 2, 512], F32, tag="ps_s", name=f"pj{nb}_{m}"
                    )
                    for k in range(KT_TILES):
                        for a in range(2):
                            nc.tensor.matmul(
                                pj[:, a, :],
                                w_in_sb[:, k, m * 128:(m + 1) * 128],
                                xin[:, k, a, :],
                                start=(k == 0),
                                stop=(k == KT_TILES - 1),
                            )
                    pjf = pj.rearrange("p a b -> p (a b)")
                    if m == 0:
                        nc.vector.tensor_copy(QT[:, ncol], pjf)
                    elif m == 1:
                        nc.vector.tensor_copy(KT[:, ncol], pjf)
                    else:
                        vstage = strm.tile([128, 1024], BF16, tag="vstage")
                        nc.vector.tensor_copy(vstage, pjf)
                        for c in range(8):
                            g = nb * 8 + c
                            tp = ps.tile(
                                [128, 128], BF16, tag="ps_o", bufs=2, name=f"tp{g}"
                            )
                            nc.tensor.transpose(
                                tp, vstage[:, c * 128:(c + 1) * 128], ident
                            )
                            nc.vector.tensor_copy(Vt[:, g, 0:64], tp[:, 0:64])
                            nc.vector.tensor_copy(
                                Vt[:, g, 65:129], tp[:, 64:128]
                            )

            # ================= Phase 2: attention (heads interleaved) ======
            for b in range(B):
                for ihalf in range(IH):
                    i0 = b * N + ihalf * 1024
                    icol = slice(i0, i0 + 1024)
                    po_h = [
                        ps.tile(
                            [65, 2, 512], F32, tag="ps_o", bufs=2,
                            name=f"po{b}_{ihalf}_{h}",
                        )
                        for h in range(HPC)
                    ]
                    for jt in range(JT):
                        jcol = slice(b * N + jt * 128, b * N + jt * 128 + 128)
                        for h in range(HPC):
                            hp = slice(h * 64, (h + 1) * 64)
                            st = ps.tile(
                                [128, 2, 512], F32, tag="ps_s",
                                name=f"st{b}_{ihalf}_{jt}_{h}",
                            )
                            for a in range(2):
                                nc.tensor.matmul(
                                    st[:, a, :], KT[hp, jcol],
                                    QT[hp, i0 + a * 512:i0 + (a + 1) * 512],
                                    start=True, stop=True,
                                )
                            et = etp.tile(
                                [128, 1024], BF16, tag="et", name="et"
                            )
                            nc.scalar.activation(
                                et, st.rearrange("p a b -> p (a b)"),
                                mybir.ActivationFunctionType.Exp,
                                scale=SCALE,
                            )
                            for a in range(2):
                                nc.tensor.matmul(
                                    po_h[h][:, a, :],
                                    Vt[:, b * JT + jt, h * 65:h * 65 + 65],
                                    et[:, a * 512:(a + 1) * 512],
                                    start=(jt == 0), stop=(jt == JT - 1),
                                )
                    # drain psum: o rows -> o_sb (unnormalized), denom -> DRAM
                    for h in range(HPC):
                        seg = b * 4 + h * 2 + ihalf
                        po_f = po_h[h].rearrange("p a b -> p (a b)")
                        if h == 0:
                            nc.vector.tensor_copy(o_sb[0:64, icol], po_f[0:64, :])
                        else:
                            h1s = strm.tile([64, 1024], BF16, tag="h1s")
                            nc.vector.tensor_copy(h1s, po_f[0:64, :])
                            nc.sync.dma_start(out=o_sb[64:128, icol], in_=h1s)
                        dnst = strm.tile([1, 1024], F32, tag="dnst")
                        nc.vector.tensor_copy(dnst, po_f[64:65, :])
                        nc.sync.dma_start(
                            out=dn_dram[seg:seg + 1, :], in_=dnst[0:1, :]
                        )

                # -------- normalize batch b (overlaps next batch) --------
                dns = strm.tile([4, 1024], F32, tag="dns")
                nc.sync.dma_start(out=dns, in_=dn_dram[b * 4:(b + 1) * 4, :])
                with nc.allow_low_precision(reason="denom broadcast"):
                    nc.vector.reciprocal(dns, dns)
                nc.sync.dma_start(out=rc_dram[b * 4:(b + 1) * 4, :], in_=dns)
                for h in range(HPC):
                    for ihalf in range(IH):
                        seg = b * 4 + h * 2 + ihalf
                        i0 = b * N + ihalf * 1024
                        icol = slice(i0, i0 + 1024)
                        rows = slice(h * 64, (h + 1) * 64)
                        bcast = strm.tile([128, 1024], F32, tag="bcast")
                        src = rc_dram[seg:seg + 1, :]
                        rbc = bass.AP(
                            tensor=src.tensor,
                            offset=src.offset,
                            ap=[[0, 64]] + list(src.ap)[1:],
                        )
                        nc.sync.dma_start(out=bcast[rows, :], in_=rbc)
                        nc.vector.tensor_mul(
                            o_sb[rows, icol], o_sb[rows, icol], bcast[rows, :]
                        )

            # ================= Phase 3: output projection =================
            for mt in range(DIM // 128):
                for nb2 in range(NT // 1024):
                    pp = ps.tile(
                        [128, 2, 512], F32, tag="ps_s", name=f"pp{mt}_{nb2}"
                    )
                    for a in range(2):
                        nc.tensor.matmul(
                            pp[:, a, :],
                            w_out_sb[:, mt * 128:(mt + 1) * 128],
                            o_sb[
                                :,
                                nb2 * 1024 + a * 512:nb2 * 1024 + (a + 1) * 512,
                            ],
                            start=True, stop=True,
                        )
                    pout = strm.tile([128, 1024], F32, tag="pout", bufs=4)
                    if nb2 % 2 == 0:
                        nc.vector.tensor_copy(
                            pout, pp.rearrange("p a b -> p (a b)")
                        )
                    else:
                        nc.scalar.copy(pout, pp.rearrange("p a b -> p (a b)"))
                    nc.sync.dma_start(
                        out=po[
                            mt * 128:(mt + 1) * 128,
                            nb2 * 1024:(nb2 + 1) * 1024,
                        ],
                        in_=pout,
                    )

    nc.finalize()
    return nc


_CACHED = {}


def kernel(x, w_in, w_out, b_out, _trace=False):
    if "nc" not in _CACHED:
        _CACHED["nc"] = _build_nc()
    nc = _CACHED["nc"]

    x2 = np.ascontiguousarray(
        x.reshape(NT, DIM).T.astype(np.float32)
    )  # [DIM, NT]
    in_maps = []
    for c in range(NCORES):
        h0, h1 = HPC * c, HPC * c + 1
        cols = []
        for part in range(3):  # q, k, v
            base = part * DIM
            cols.extend(range(base + h0 * HD, base + h0 * HD + HD))
            cols.extend(range(base + h1 * HD, base + h1 * HD + HD))
        w_in_c = np.ascontiguousarray(w_in[:, cols].astype(np.float32))
        w_out_c = np.ascontiguousarray(
            w_out[128 * c:128 * (c + 1), :].astype(np.float32)
        )
        in_maps.append(
            {
                "xT": x2.astype(ml_dtypes.bfloat16),
                "w_in_c": w_in_c.astype(ml_dtypes.bfloat16),
                "w_out_c": w_out_c.astype(ml_dtypes.bfloat16),
            }
        )

    res = run_bass_kernel_spmd(
        nc, in_maps, core_ids=list(range(NCORES)), trace=_trace
    )
    acc = res.results[0]["po"].astype(np.float64)
    for c in range(1, NCORES):
        acc = acc + res.results[c]["po"].astype(np.float64)
    out = acc.T + b_out.astype(np.float64)
    if _trace:
        kernel.last_result = res
    return np.ascontiguousarray(out.reshape(B, N, DIM).astype(np.float32))
